# revision 1
# baseline (speedup 1.0000x reference)
"""Causal self-attention (B=4, T=2048, C=2048, H=16) on 8 trn2 NeuronCores.

Sharding: tensor-parallel over heads (2 heads/core). Each core computes the
QKV projection for its head shard (q,k produced transposed for the scores
matmul, v produced in normal layout for attn@v), applies rope fused into the
PSUM->SBUF drain, runs causal attention without max-subtraction (scores are
O(5), exp is fp32-safe), and produces yT = (attn @ v)^T per head. An
AllToAll re-shards Y^T from head-column-sharded to token-row-sharded, after
which each core row-shards the output projection against the full Wproj and
returns its 1024-row chunk of the output.

Matmul dtype is switchable: bf16 (fast-weight-load, 1 cyc/row) or fp32r
(single-pass fp32, ~2 cyc/row + exposed LDWEIGHTS). Softmax denominator is
computed on the PE as an accumulating ones-column matmul; causal masking is
a gpsimd affine_select zero-fill on the exp output; reciprocals are batched
to one [8, 512] DVE op per batch.
"""

import os
import sys

os.environ.setdefault("JAX_PLATFORMS", "axon")

import numpy as np

B, T, C = 4, 2048, 2048
H = 16
HD = 128
N_CORES = 8
HL = H // N_CORES  # heads per core = 2
CL = HL * HD  # per-core head columns = 256
TQ = 512  # Tq chunk for scores
NKT = T // 128  # 16 tiles of 128 along T
KC = C // 128  # 16 k-tiles along C
ROWS = B * T // N_CORES  # output rows per core = 1024

MM_DTYPE = os.environ.get("KERNEL_MM_DTYPE", "f32r")  # "bf16" | "f32r"


def _install_ntff_shim():
    """The agent image's antenv lacks axon_hooks; provide it so
    run_bass_kernel_spmd(trace=True) can reach the NTFF profiler."""
    import types, contextlib, ctypes

    try:
        from antenv.axon_hooks import get_axon_ntff_profile_hook  # noqa

        return
    except ImportError:
        pass

    so_path = "/opt/axon/libaxon_pjrt.so"
    try:
        lib = ctypes.CDLL(so_path)
    except OSError:
        lib = None
    if lib is None or not hasattr(lib, "axon_start_nrt_profile"):
        hook = None
    else:
        lib.axon_start_nrt_profile.argtypes = [
            ctypes.POINTER(ctypes.c_int64),
            ctypes.c_size_t,
        ]
        lib.axon_start_nrt_profile.restype = ctypes.c_int64
        lib.axon_stop_nrt_profile.argtypes = [ctypes.c_char_p]
        lib.axon_stop_nrt_profile.restype = ctypes.c_int64

        @contextlib.contextmanager
        def hook(output_dir, device_ids):
            import jax

            jax.devices()
            if device_ids:
                ids = (ctypes.c_int64 * len(device_ids))(*device_ids)
                rc = lib.axon_start_nrt_profile(ids, len(device_ids))
            else:
                rc = lib.axon_start_nrt_profile(None, 0)
            if rc != 0:
                raise RuntimeError(f"axon_start_nrt_profile rc={rc}")
            try:
                yield
            finally:
                n = lib.axon_stop_nrt_profile(str(output_dir).encode())
                if n <= 0:
                    print(f"ntff profile: rc={n} (no files) dir={output_dir}")

    import antenv

    mod = types.ModuleType("antenv.axon_hooks")
    _state = {"hook": hook}
    mod.set_axon_ntff_profile_hook = lambda h: _state.__setitem__("hook", h)
    mod.get_axon_ntff_profile_hook = lambda: _state["hook"]
    sys.modules["antenv.axon_hooks"] = mod
    antenv.axon_hooks = mod


def build_program():
    import concourse.bass as bass
    import concourse.mybir as mybir
    import concourse.tile as tile
    from concourse import bacc
    from contextlib import ExitStack

    f32 = mybir.dt.float32
    f32r = mybir.dt.float32r
    bf16 = mybir.dt.bfloat16
    mdt = bf16 if MM_DTYPE == "bf16" else f32r
    Exp = mybir.ActivationFunctionType.Exp

    nc = bacc.Bacc("TRN2", target_bir_lowering=False, debug=False, num_devices=N_CORES)

    xT = nc.dram_tensor("xT", [B, C, T], mdt, kind="ExternalInput")
    wqk = nc.dram_tensor("wqk", [C, 4 * HD], mdt, kind="ExternalInput")
    wv = nc.dram_tensor("wv", [C, CL], mdt, kind="ExternalInput")
    wproj = nc.dram_tensor("wproj", [C, C], mdt, kind="ExternalInput")
    cosd = nc.dram_tensor("cos", [HD // 2, T], f32, kind="ExternalInput")
    sind = nc.dram_tensor("sin", [HD // 2, T], f32, kind="ExternalInput")
    seld = nc.dram_tensor(
        "sel", [2 * (T // TQ), 2 * (T // TQ) * 128], f32r, kind="ExternalInput"
    )
    out = nc.dram_tensor("out", [ROWS, C], f32, kind="ExternalOutput")

    # A2A runs on a plain dtype; in f32r mode declare f32 and bitcast at the
    # endpoints (values are already f32r-rounded by the producing DVE op).
    # Split in two along the per-rank t axis so the second transfer overlaps
    # the first half of the output projection.
    a2a_dt = bf16 if mdt == bf16 else f32
    HALF = ROWS // 2  # 512
    a2a_in = [
        nc.dram_tensor(f"a2a_in{k}", [N_CORES, CL, HALF], a2a_dt) for k in range(2)
    ]
    a2a_out = [
        nc.dram_tensor(f"a2a_out{k}", [N_CORES, CL, HALF], a2a_dt) for k in range(2)
    ]

    wqk_t = wqk[:, :].rearrange("(ko p) m -> p ko m", p=128)  # [128, KC, 512]
    wv_t = wv[:, :].rearrange("(ko p) m -> p ko m", p=128)  # [128, KC, 256]
    wproj_t = wproj[:, :].rearrange("(ko p) n -> p ko n", p=128)  # [128, KC, 2048]
    # a2a_out rows (src_core, l) flatten to the global Y column index; view as
    # [p, kt, t] k-tiles for the proj lhsT.
    yt_t = []
    for k in range(2):
        v = a2a_out[k][:, :, :].rearrange("s (lh p) t -> p (s lh) t", p=128)
        if mdt != bf16:
            v = v.bitcast(f32r)
        yt_t.append(v)

    with tile.TileContext(nc) as tc:
        with ExitStack() as top:
            const = top.enter_context(tc.tile_pool(name="const", bufs=1))
            wpool = top.enter_context(tc.tile_pool(name="weights", bufs=1))

            # --- constants ---
            ones_col_f = const.tile([128, 1], f32, tag="ones_col_f")
            nc.vector.memset(ones_col_f[:], 1.0)
            ones_col = const.tile([128, 1], mdt, tag="ones_col")
            nc.vector.tensor_copy(ones_col[:], ones_col_f[:])
            # selector matrix (host-built): sel[:, idx*128:(idx+1)*128] has
            # row idx all-ones -> matmul(sel[idx].T @ rec_all) broadcasts
            # rec_all[idx] to 128 partitions.
            NCH = 2 * (T // TQ)  # chunks per batch = 8
            sel = const.tile([NCH, NCH * 128], f32r, tag="sel")
            nc.sync.dma_start(sel[:], seld[:, :])

            cos_sb = const.tile([64, T], f32, tag="cos")
            nc.sync.dma_start(cos_sb[:], cosd[:, :])
            sin_sb = const.tile([64, T], f32, tag="sin")
            nc.sync.dma_start(sin_sb[:], sind[:, :])

            # --- weights resident in SBUF ---
            wqk_sb = wpool.tile([128, KC, 4 * HD], mdt, tag="wqk")
            nc.sync.dma_start(wqk_sb[:], wqk_t)
            wv_sb = wpool.tile([128, KC, CL], mdt, tag="wv")
            nc.sync.dma_start(wv_sb[:], wv_t)

            with ExitStack() as mid:
                big = mdt == bf16  # f32r tiles are 2x, trim bufs to fit SBUF
                qk_pool = mid.enter_context(
                    tc.tile_pool(name="qkT", bufs=2 if big else 1)
                )
                v_pool = mid.enter_context(
                    tc.tile_pool(name="vsb", bufs=2 if big else 1)
                )
                xk_pool = mid.enter_context(
                    tc.tile_pool(name="xk", bufs=20 if big else 17)
                )
                rtmp = mid.enter_context(
                    tc.tile_pool(name="rtmp", bufs=2 if big else 1)
                )
                apool = mid.enter_context(
                    tc.tile_pool(name="apool", bufs=3 if big else 2)
                )
                spool = mid.enter_context(tc.tile_pool(name="spool", bufs=2))
                spool1 = mid.enter_context(tc.tile_pool(name="spool1", bufs=1))
                ps2 = mid.enter_context(tc.tile_pool(name="ps2", bufs=2, space="PSUM"))
                ps1 = mid.enter_context(tc.tile_pool(name="ps1", bufs=1, space="PSUM"))

                for b in range(B):
                    # ---------- QKV projection for batch b ----------
                    # qkT [128, 4, T]: m=0,1 -> qT heads 0,1 (rope+scale),
                    # m=2,3 -> kT heads 0,1 (rope). v_sb [128, NKT, CL].
                    qkT = qk_pool.tile([128, 4, T], mdt, tag="qkT")
                    v_sb = v_pool.tile([128, NKT, CL], mdt, tag="v")

                    for n in range(T // TQ):
                        xk = [
                            xk_pool.tile([128, TQ], mdt, tag="xk", name=f"xk{k}")
                            for k in range(KC)
                        ]
                        for k in range(KC):
                            nc.sync.dma_start(
                                xk[k][:],
                                xT[b, 128 * k : 128 * (k + 1), TQ * n : TQ * (n + 1)],
                            )
                        for m in range(4):
                            qk_ps = ps2.tile([128, TQ], f32, tag="qk")
                            for k in range(KC):
                                nc.tensor.matmul(
                                    qk_ps[:],
                                    wqk_sb[:, k, 128 * m : 128 * (m + 1)],
                                    xk[k][:],
                                    start=(k == 0),
                                    stop=(k == KC - 1),
                                )
                            # rope on the PSUM->SBUF drain
                            cos_t = cos_sb[:, TQ * n : TQ * (n + 1)]
                            sin_t = sin_sb[:, TQ * n : TQ * (n + 1)]
                            t0 = rtmp.tile([64, TQ], f32, tag="t0")
                            t1 = rtmp.tile([64, TQ], f32, tag="t1")
                            nc.vector.tensor_mul(t0[:], qk_ps[0:64, :], cos_t)
                            nc.vector.tensor_mul(t1[:], qk_ps[64:128, :], sin_t)
                            nc.vector.tensor_sub(
                                qkT[0:64, m, TQ * n : TQ * (n + 1)], t0[:], t1[:]
                            )
                            t2 = rtmp.tile([64, TQ], f32, tag="t2")
                            t3 = rtmp.tile([64, TQ], f32, tag="t3")
                            nc.vector.tensor_mul(t2[:], qk_ps[64:128, :], cos_t)
                            nc.vector.tensor_mul(t3[:], qk_ps[0:64, :], sin_t)
                            nc.vector.tensor_add(
                                qkT[64:128, m, TQ * n : TQ * (n + 1)], t2[:], t3[:]
                            )
                        for m2 in range(4):
                            v_ps = ps1.tile([128, CL], f32, tag="v")
                            for k in range(KC):
                                nc.tensor.matmul(
                                    v_ps[:],
                                    xk[k][:, 128 * m2 : 128 * (m2 + 1)],
                                    wv_sb[:, k, :],
                                    start=(k == 0),
                                    stop=(k == KC - 1),
                                )
                            nc.scalar.copy(v_sb[:, 4 * n + m2, :], v_ps[:])

                    # ---------- attention for batch b ----------
                    den_all = spool1.tile([2 * (T // TQ), TQ], f32, tag="den_all")
                    yraw = [None] * (2 * (T // TQ))
                    for h in range(HL):
                        for j in range(T // TQ):
                            idx = h * (T // TQ) + j
                            ntk = 4 * j + 4  # causal: k-tiles 0..4j+3
                            yT_ps = ps1.tile([128, TQ], f32, tag="yT")
                            den_ps = ps1.tile([1, TQ], f32, tag="den")
                            for i in range(ntk):
                                sT_ps = ps2.tile([128, TQ], f32, tag="sT")
                                nc.tensor.matmul(
                                    sT_ps[:],
                                    qkT[:, 2 + h, 128 * i : 128 * (i + 1)],
                                    qkT[:, h, TQ * j : TQ * (j + 1)],
                                    start=True,
                                    stop=True,
                                )
                                a_sb = apool.tile([128, TQ], mdt, tag="a")
                                nc.scalar.activation(a_sb[:], sT_ps[:], Exp)
                                d = i - 4 * j
                                if d >= 0:
                                    # causal: zero where tq < tk (y < x + 128d)
                                    nc.gpsimd.affine_select(
                                        out=a_sb[:],
                                        in_=a_sb[:],
                                        compare_op=mybir.AluOpType.is_ge,
                                        fill=0.0,
                                        base=-128 * d,
                                        pattern=[[1, TQ]],
                                        channel_multiplier=-1,
                                    )
                                nc.tensor.matmul(
                                    den_ps[:],
                                    ones_col[:],
                                    a_sb[:],
                                    start=(i == 0),
                                    stop=(i == ntk - 1),
                                )
                                nc.tensor.matmul(
                                    yT_ps[:],
                                    v_sb[:, i, 128 * h : 128 * (h + 1)],
                                    a_sb[:],
                                    start=(i == 0),
                                    stop=(i == ntk - 1),
                                )
                            dtmp = spool.tile([1, TQ], f32, tag="dtmp")
                            nc.vector.tensor_copy(dtmp[:], den_ps[:])
                            nc.sync.dma_start(den_all[idx : idx + 1, :], dtmp[:])
                            yr = spool1.tile(
                                [128, TQ], f32, tag=f"yraw{idx}", name=f"yr{idx}"
                            )
                            nc.scalar.copy(yr[:], yT_ps[:])
                            yraw[idx] = yr

                    # batched reciprocal + per-chunk broadcast and normalize
                    rec_all = spool1.tile([2 * (T // TQ), TQ], f32r, tag="rec_all")
                    with nc.allow_low_precision(reason="softmax denom recip"):
                        nc.vector.reciprocal(rec_all[:], den_all[:])
                    for h in range(HL):
                        for j in range(T // TQ):
                            idx = h * (T // TQ) + j
                            bc_ps = ps1.tile([128, TQ], f32, tag="bc")
                            nc.tensor.matmul(
                                bc_ps[:],
                                sel[:, idx * 128 : (idx + 1) * 128],
                                rec_all[:],
                                start=True,
                                stop=True,
                            )
                            yT_sb = spool.tile([128, TQ], mdt, tag="yT_sb")
                            nc.vector.tensor_mul(yT_sb[:], yraw[idx][:], bc_ps[:])
                            dest = 2 * b + (1 if TQ * j >= ROWS else 0)
                            half = j % 2  # ((TQ*j) % ROWS) // HALF
                            src = yT_sb[:]
                            if mdt != bf16:
                                src = src.bitcast(f32)
                            nc.sync.dma_start(
                                a2a_in[half][dest, 128 * h : 128 * (h + 1), :],
                                src,
                            )

            # ---------- all-to-all: head-sharded Y^T -> row-sharded Y^T ----------
            # Two half-width collectives; proj on half 0 overlaps transfer of
            # half 1.
            for k in range(2):
                nc.gpsimd.collective_compute(
                    "AllToAll",
                    mybir.AluOpType.bypass,
                    replica_groups=[list(range(N_CORES))],
                    ins=[a2a_in[k][:, :, :]],
                    outs=[a2a_out[k][:, :, :]],
                )

            # ---------- output projection (row-sharded) ----------
            with ExitStack() as pj:
                ypool = pj.enter_context(tc.tile_pool(name="yproj", bufs=1))
                wp_pool = pj.enter_context(tc.tile_pool(name="wpr", bufs=2))
                opool = pj.enter_context(tc.tile_pool(name="osb", bufs=3))
                ps_o = pj.enter_context(tc.tile_pool(name="pso", bufs=4, space="PSUM"))

                for half in range(2):
                    y_sb = ypool.tile(
                        [128, KC, HALF], mdt, tag=f"y{half}", name=f"y{half}"
                    )
                    nc.sync.dma_start(y_sb[:], yt_t[half])
                    for n in range(C // TQ):
                        wp_sb = wp_pool.tile([128, KC, TQ], mdt, tag="wp")
                        nc.sync.dma_start(
                            wp_sb[:], wproj_t[:, :, TQ * n : TQ * (n + 1)]
                        )
                        for m in range(HALF // 128):
                            mg = half * (HALF // 128) + m
                            o_ps = ps_o.tile([128, TQ], f32, tag="o")
                            for k in range(KC):
                                nc.tensor.matmul(
                                    o_ps[:],
                                    y_sb[:, k, 128 * m : 128 * (m + 1)],
                                    wp_sb[:, k, :],
                                    start=(k == 0),
                                    stop=(k == KC - 1),
                                )
                            o_sb = opool.tile([128, TQ], f32, tag="o_sb")
                            nc.scalar.copy(o_sb[:], o_ps[:])
                            nc.sync.dma_start(
                                out[128 * mg : 128 * (mg + 1), TQ * n : TQ * (n + 1)],
                                o_sb[:],
                            )

    nc.compile()
    return nc


_PERM = None


def _prep_inputs(x, rope, Wqkv, Wproj):
    """Host-side sharding/layout prep (numpy only)."""
    global _PERM
    if _PERM is None:
        _PERM = np.concatenate([np.arange(0, HD, 2), np.arange(1, HD, 2)])
    perm = _PERM

    if MM_DTYPE == "bf16":
        import ml_dtypes

        mdt_np = ml_dtypes.bfloat16
    else:
        mdt_np = np.float32

    x = np.asarray(x, dtype=np.float32)
    xT = np.ascontiguousarray(x.transpose(0, 2, 1)).astype(mdt_np)  # [B, C, T]

    rope = np.asarray(rope, dtype=np.float32)
    cos = np.ascontiguousarray(rope[:, :, 0].T)  # [64, T]
    sin = np.ascontiguousarray(rope[:, :, 1].T)

    Wqkv = np.asarray(Wqkv, dtype=np.float32)
    Wq = Wqkv[:, 0:C]
    Wk = Wqkv[:, C : 2 * C]
    Wv = Wqkv[:, 2 * C : 3 * C]
    scale = 1.0 / np.sqrt(HD)
    Wproj_m = np.ascontiguousarray(np.asarray(Wproj, dtype=np.float32)).astype(mdt_np)

    NCH = 2 * (T // TQ)
    sel_np = np.zeros((NCH, NCH * 128), dtype=np.float32)
    for idx in range(NCH):
        sel_np[idx, idx * 128 : (idx + 1) * 128] = 1.0

    in_maps = []
    for c in range(N_CORES):
        cols = []
        for lh in range(HL):
            h = HL * c + lh
            cols.append(h * HD + perm)
        qcols = np.concatenate(cols)
        wq_c = Wq[:, qcols] * scale
        wk_c = Wk[:, qcols]
        wqk_c = np.ascontiguousarray(
            np.concatenate([wq_c, wk_c], axis=1)
        ).astype(mdt_np)  # [C, 512]
        wv_c = np.ascontiguousarray(
            Wv[:, HL * HD * c : HL * HD * (c + 1)]
        ).astype(mdt_np)  # [C, 256]
        in_maps.append(
            {
                "xT": xT,
                "wqk": wqk_c,
                "wv": wv_c,
                "wproj": Wproj_m,
                "cos": cos,
                "sin": sin,
                "sel": sel_np,
            }
        )
    return in_maps


_NC_CACHE = None


def _get_nc():
    global _NC_CACHE
    if _NC_CACHE is None:
        _NC_CACHE = build_program()
    return _NC_CACHE


def run(x, rope, Wqkv, Wproj, trace=False):
    _install_ntff_shim()
    from concourse.bass_utils import run_bass_kernel_spmd

    nc = _get_nc()
    in_maps = _prep_inputs(x, rope, Wqkv, Wproj)
    res = run_bass_kernel_spmd(nc, in_maps, list(range(N_CORES)), trace=trace)
    chunks = [res.results[c]["out"] for c in range(N_CORES)]
    full = np.concatenate(chunks, axis=0).reshape(B, T, C)
    return full, res


def kernel(x, rope, Wqkv, Wproj):
    out, _ = run(x, rope, Wqkv, Wproj, trace=False)
    return out


if __name__ == "__main__":
    import time

    t0 = time.time()
    nc = build_program()
    ni = sum(len(bb.instructions) for f in nc.m.functions for bb in f.blocks)
    print(f"build ok ({MM_DTYPE}): {time.time()-t0:.1f}s, {ni} instructions")



# revision 7
# speedup vs baseline: 1.0920x; 1.0920x over previous
"""Causal self-attention (B=4, T=2048, C=2048, H=16) on 8 trn2 NeuronCores.

Sharding: tensor-parallel over heads (2 heads/core). Each core computes the
QKV projection for its head shard (q,k produced transposed for the scores
matmul, v produced in normal layout for attn@v), applies rope fused into the
PSUM->SBUF drain, runs causal attention without max-subtraction (scores are
O(5), exp is fp32-safe), and produces yT = (attn @ v)^T per head.

All matmuls are bf16 (fast-weight-load path, 1 cyc/row). The AllToAll that
re-shards Y^T from head-column-sharded to token-row-sharded is split into
FOUR per-batch collectives, each fired as soon as that batch's attention is
done; the output projection for batch b is emitted during batch b+1's
compute so the collective latency is hidden. Each core ends up with tokens
[c*256, (c+1)*256) of every batch (re-assembled on host). Wproj is resident
in SBUF for the whole kernel.

Softmax: denominator accumulated on the PE as a ones-column matmul; per
chunk the reciprocal is broadcast to 128 partitions with a tiny ones-row
matmul and the normalize happens on the DVE straight out of PSUM."""

import os
import sys

os.environ.setdefault("JAX_PLATFORMS", "axon")

import numpy as np

B, T, C = 4, 2048, 2048
H = 16
HD = 128
N_CORES = 8
HL = H // N_CORES  # heads per core = 2
CL = HL * HD  # per-core head columns = 256
TQ = 512  # Tq chunk for scores
NKT = T // 128  # 16 tiles of 128 along T
KC = C // 128  # 16 k-tiles along C
TB = T // N_CORES  # tokens per (batch, core) after re-shard = 256
ROWS = B * TB  # output rows per core = 1024


def _install_ntff_shim():
    """The agent image's antenv lacks axon_hooks; provide it so
    run_bass_kernel_spmd(trace=True) can reach the NTFF profiler."""
    import types, contextlib, ctypes

    try:
        from antenv.axon_hooks import get_axon_ntff_profile_hook  # noqa

        return
    except ImportError:
        pass

    so_path = "/opt/axon/libaxon_pjrt.so"
    try:
        lib = ctypes.CDLL(so_path)
    except OSError:
        lib = None
    if lib is None or not hasattr(lib, "axon_start_nrt_profile"):
        hook = None
    else:
        lib.axon_start_nrt_profile.argtypes = [
            ctypes.POINTER(ctypes.c_int64),
            ctypes.c_size_t,
        ]
        lib.axon_start_nrt_profile.restype = ctypes.c_int64
        lib.axon_stop_nrt_profile.argtypes = [ctypes.c_char_p]
        lib.axon_stop_nrt_profile.restype = ctypes.c_int64

        @contextlib.contextmanager
        def hook(output_dir, device_ids):
            import jax

            jax.devices()
            if device_ids:
                ids = (ctypes.c_int64 * len(device_ids))(*device_ids)
                rc = lib.axon_start_nrt_profile(ids, len(device_ids))
            else:
                rc = lib.axon_start_nrt_profile(None, 0)
            if rc != 0:
                raise RuntimeError(f"axon_start_nrt_profile rc={rc}")
            try:
                yield
            finally:
                n = lib.axon_stop_nrt_profile(str(output_dir).encode())
                if n <= 0:
                    print(f"ntff profile: rc={n} (no files) dir={output_dir}")

    import antenv

    mod = types.ModuleType("antenv.axon_hooks")
    _state = {"hook": hook}
    mod.set_axon_ntff_profile_hook = lambda h: _state.__setitem__("hook", h)
    mod.get_axon_ntff_profile_hook = lambda: _state["hook"]
    sys.modules["antenv.axon_hooks"] = mod
    antenv.axon_hooks = mod


def build_program():
    import concourse.bass as bass
    import concourse.mybir as mybir
    import concourse.tile as tile
    from concourse import bacc
    from contextlib import ExitStack

    f32 = mybir.dt.float32
    f32r = mybir.dt.float32r
    bf16 = mybir.dt.bfloat16
    mdt = bf16
    Exp = mybir.ActivationFunctionType.Exp

    nc = bacc.Bacc("TRN2", target_bir_lowering=False, debug=False, num_devices=N_CORES)

    xT = nc.dram_tensor("xT", [B, C, T], mdt, kind="ExternalInput")
    wqk = nc.dram_tensor("wqk", [C, 4 * HD], mdt, kind="ExternalInput")
    wv = nc.dram_tensor("wv", [C, CL], mdt, kind="ExternalInput")
    wproj = nc.dram_tensor("wproj", [C, C], mdt, kind="ExternalInput")
    cosd = nc.dram_tensor("cos", [HD // 2, T], f32, kind="ExternalInput")
    sind = nc.dram_tensor("sin", [HD // 2, T], f32, kind="ExternalInput")
    out = nc.dram_tensor("out", [ROWS, C], f32, kind="ExternalOutput")

    a2a_in = [
        nc.dram_tensor(f"a2a_in{b}", [N_CORES, CL, TB], mdt) for b in range(B)
    ]
    a2a_out = [
        nc.dram_tensor(f"a2a_out{b}", [N_CORES, CL, TB], mdt) for b in range(B)
    ]

    wqk_t = wqk[:, :].rearrange("(ko p) m -> p ko m", p=128)  # [128, KC, 512]
    wv_t = wv[:, :].rearrange("(ko p) m -> p ko m", p=128)  # [128, KC, 256]
    wproj_t = wproj[:, :].rearrange("(ko p) n -> p ko n", p=128)  # [128, KC, 2048]
    # a2a_out rows (src_core, l) flatten to the global Y column index; view as
    # [p, kt, t] k-tiles for the proj lhsT.
    yt_t = [
        a2a_out[b][:, :, :].rearrange("s (lh p) t -> p (s lh) t", p=128)
        for b in range(B)
    ]

    with tile.TileContext(nc) as tc:
        with ExitStack() as top:
            const = top.enter_context(tc.tile_pool(name="const", bufs=1))
            wpool = top.enter_context(tc.tile_pool(name="weights", bufs=1))

            # --- constants ---
            ones_col_f = const.tile([128, 1], f32, tag="ones_col_f")
            nc.vector.memset(ones_col_f[:], 1.0)
            ones_col = const.tile([128, 1], mdt, tag="ones_col")
            nc.vector.tensor_copy(ones_col[:], ones_col_f[:])

            cos_sb = const.tile([64, T], f32, tag="cos")
            nc.sync.dma_start(cos_sb[:], cosd[:, :])
            sin_sb = const.tile([64, T], f32, tag="sin")
            nc.sync.dma_start(sin_sb[:], sind[:, :])

            # --- weights resident in SBUF ---
            wqk_sb = wpool.tile([128, KC, 4 * HD], mdt, tag="wqk")
            nc.sync.dma_start(wqk_sb[:], wqk_t)
            wv_sb = wpool.tile([128, KC, CL], mdt, tag="wv")
            nc.sync.dma_start(wv_sb[:], wv_t)
            # wproj via the scalar queue so it doesn't delay the xk stream
            wproj_sb = wpool.tile([128, KC, C], mdt, tag="wproj")
            nc.scalar.dma_start(wproj_sb[:], wproj_t)

            with ExitStack() as mid:
                qk_pool = mid.enter_context(tc.tile_pool(name="qkT", bufs=2))
                v_pool = mid.enter_context(tc.tile_pool(name="vsb", bufs=2))
                xk_pool = mid.enter_context(tc.tile_pool(name="xk", bufs=18))
                rtmp = mid.enter_context(tc.tile_pool(name="rtmp", bufs=1))
                apool = mid.enter_context(tc.tile_pool(name="apool", bufs=3))
                spool = mid.enter_context(tc.tile_pool(name="spool", bufs=2))
                ypool = mid.enter_context(tc.tile_pool(name="yproj", bufs=1))
                opool = mid.enter_context(tc.tile_pool(name="osb", bufs=2))
                ps_qk = mid.enter_context(
                    tc.tile_pool(name="psqk", bufs=2, space="PSUM")
                )
                ps_sT = mid.enter_context(
                    tc.tile_pool(name="pssT", bufs=2, space="PSUM")
                )
                ps_yv = mid.enter_context(
                    tc.tile_pool(name="psyv", bufs=1, space="PSUM")
                )
                ps_db = mid.enter_context(
                    tc.tile_pool(name="psdb", bufs=1, space="PSUM")
                )
                ps_o = mid.enter_context(
                    tc.tile_pool(name="pso", bufs=1, space="PSUM")
                )

                def emit_proj(b, y_sb):
                    for n in range(C // TQ):
                        for m in range(TB // 128):
                            o_ps = ps_o.tile([128, TQ], f32, tag="o")
                            for k in range(KC):
                                nc.tensor.matmul(
                                    o_ps[:],
                                    y_sb[:, k, 128 * m : 128 * (m + 1)],
                                    wproj_sb[:, k, TQ * n : TQ * (n + 1)],
                                    start=(k == 0),
                                    stop=(k == KC - 1),
                                )
                            o_sb = opool.tile([128, TQ], f32, tag="o_sb")
                            nc.scalar.copy(o_sb[:], o_ps[:])
                            nc.sync.dma_start(
                                out[
                                    TB * b + 128 * m : TB * b + 128 * (m + 1),
                                    TQ * n : TQ * (n + 1),
                                ],
                                o_sb[:],
                            )

                y_prev = None
                for b in range(B):
                    # ---------- QKV projection for batch b ----------
                    # qkT [128, 4, T]: m=0,1 -> qT heads 0,1 (rope+scale),
                    # m=2,3 -> kT heads 0,1 (rope). v_sb [128, NKT, CL].
                    qkT = qk_pool.tile([128, 4, T], mdt, tag="qkT")
                    v_sb = v_pool.tile([128, NKT, CL], mdt, tag="v")

                    for n in range(T // TQ):
                        xk = [
                            xk_pool.tile([128, TQ], mdt, tag="xk", name=f"xk{k}")
                            for k in range(KC)
                        ]
                        for k in range(KC):
                            nc.sync.dma_start(
                                xk[k][:],
                                xT[b, 128 * k : 128 * (k + 1), TQ * n : TQ * (n + 1)],
                            )
                        for m in range(4):
                            qk_ps = ps_qk.tile([128, TQ], f32, tag="qk")
                            for k in range(KC):
                                nc.tensor.matmul(
                                    qk_ps[:],
                                    wqk_sb[:, k, 128 * m : 128 * (m + 1)],
                                    xk[k][:],
                                    start=(k == 0),
                                    stop=(k == KC - 1),
                                )
                            # rope on the PSUM->SBUF drain
                            cos_t = cos_sb[:, TQ * n : TQ * (n + 1)]
                            sin_t = sin_sb[:, TQ * n : TQ * (n + 1)]
                            t0 = rtmp.tile([64, TQ], f32, tag="t0")
                            t1 = rtmp.tile([64, TQ], f32, tag="t1")
                            nc.vector.tensor_mul(t0[:], qk_ps[0:64, :], cos_t)
                            nc.vector.tensor_mul(t1[:], qk_ps[64:128, :], sin_t)
                            nc.vector.tensor_sub(
                                qkT[0:64, m, TQ * n : TQ * (n + 1)], t0[:], t1[:]
                            )
                            t2 = rtmp.tile([64, TQ], f32, tag="t2")
                            t3 = rtmp.tile([64, TQ], f32, tag="t3")
                            nc.vector.tensor_mul(t2[:], qk_ps[64:128, :], cos_t)
                            nc.vector.tensor_mul(t3[:], qk_ps[0:64, :], sin_t)
                            nc.vector.tensor_add(
                                qkT[64:128, m, TQ * n : TQ * (n + 1)], t2[:], t3[:]
                            )
                        for m2 in range(4):
                            v_ps = ps_yv.tile([128, CL], f32, tag="v")
                            for k in range(KC):
                                nc.tensor.matmul(
                                    v_ps[:],
                                    xk[k][:, 128 * m2 : 128 * (m2 + 1)],
                                    wv_sb[:, k, :],
                                    start=(k == 0),
                                    stop=(k == KC - 1),
                                )
                            nc.scalar.copy(v_sb[:, 4 * n + m2, :], v_ps[:])

                    # prefetch the previous batch's a2a result for its proj
                    # (collective b-1 finished during this batch's QKV)
                    if b > 0:
                        y_prev = ypool.tile([128, KC, TB], mdt, tag="y")
                        nc.scalar.dma_start(y_prev[:], yt_t[b - 1])

                    # ---------- attention for batch b ----------
                    for h in range(HL):
                        for j in range(T // TQ):
                            ntk = 4 * j + 4  # causal: k-tiles 0..4j+3
                            yT_ps = ps_yv.tile([128, TQ], f32, tag="yT")
                            den_ps = ps_db.tile([1, TQ], f32, tag="den")
                            for i in range(ntk):
                                sT_ps = ps_sT.tile([128, TQ], f32, tag="sT")
                                nc.tensor.matmul(
                                    sT_ps[:],
                                    qkT[:, 2 + h, 128 * i : 128 * (i + 1)],
                                    qkT[:, h, TQ * j : TQ * (j + 1)],
                                    start=True,
                                    stop=True,
                                )
                                a_sb = apool.tile([128, TQ], mdt, tag="a")
                                nc.scalar.activation(a_sb[:], sT_ps[:], Exp)
                                d = i - 4 * j
                                if d >= 0:
                                    # causal: zero where tq < tk (y < x + 128d)
                                    nc.gpsimd.affine_select(
                                        out=a_sb[:],
                                        in_=a_sb[:],
                                        compare_op=mybir.AluOpType.is_ge,
                                        fill=0.0,
                                        base=-128 * d,
                                        pattern=[[1, TQ]],
                                        channel_multiplier=-1,
                                    )
                                nc.tensor.matmul(
                                    den_ps[:],
                                    ones_col[:],
                                    a_sb[:],
                                    start=(i == 0),
                                    stop=(i == ntk - 1),
                                )
                                nc.tensor.matmul(
                                    yT_ps[:],
                                    v_sb[:, i, 128 * h : 128 * (h + 1)],
                                    a_sb[:],
                                    start=(i == 0),
                                    stop=(i == ntk - 1),
                                )
                            # chunk epilogue: recip -> broadcast -> normalize
                            rec_c = spool.tile([1, TQ], f32r, tag="rec")
                            with nc.allow_low_precision(reason="softmax recip"):
                                nc.vector.reciprocal(rec_c[:], den_ps[:])
                            bc_sb = spool.tile([128, TQ], f32, tag="bc")
                            nc.gpsimd.partition_broadcast(
                                bc_sb[:], rec_c[:].bitcast(f32)
                            )
                            yT_sb = spool.tile([128, TQ], mdt, tag="yT_sb")
                            nc.vector.tensor_mul(yT_sb[:], bc_sb[:], yT_ps[:])
                            for u in range(2):
                                nc.sync.dma_start(
                                    a2a_in[b][
                                        2 * j + u, 128 * h : 128 * (h + 1), :
                                    ],
                                    yT_sb[:, TB * u : TB * (u + 1)],
                                )

                    # proj for the previous batch overlaps this batch's tail
                    if b > 0:
                        emit_proj(b - 1, y_prev)

                    # fire this batch's re-shard as soon as its attn is done
                    nc.gpsimd.collective_compute(
                        "AllToAll",
                        mybir.AluOpType.bypass,
                        replica_groups=[list(range(N_CORES))],
                        ins=[a2a_in[b][:, :, :]],
                        outs=[a2a_out[b][:, :, :]],
                    )

                # ---------- last batch's projection ----------
                y_last = ypool.tile([128, KC, TB], mdt, tag="y")
                nc.scalar.dma_start(y_last[:], yt_t[B - 1])
                emit_proj(B - 1, y_last)

    nc.compile()
    return nc


_PERM = None


def _prep_inputs(x, rope, Wqkv, Wproj):
    """Host-side sharding/layout prep (numpy only)."""
    global _PERM
    if _PERM is None:
        _PERM = np.concatenate([np.arange(0, HD, 2), np.arange(1, HD, 2)])
    perm = _PERM

    import ml_dtypes

    mdt_np = ml_dtypes.bfloat16

    x = np.asarray(x, dtype=np.float32)
    xT = np.ascontiguousarray(x.transpose(0, 2, 1)).astype(mdt_np)  # [B, C, T]

    rope = np.asarray(rope, dtype=np.float32)
    cos = np.ascontiguousarray(rope[:, :, 0].T)  # [64, T]
    sin = np.ascontiguousarray(rope[:, :, 1].T)

    Wqkv = np.asarray(Wqkv, dtype=np.float32)
    Wq = Wqkv[:, 0:C]
    Wk = Wqkv[:, C : 2 * C]
    Wv = Wqkv[:, 2 * C : 3 * C]
    scale = 1.0 / np.sqrt(HD)
    Wproj_m = np.ascontiguousarray(np.asarray(Wproj, dtype=np.float32)).astype(mdt_np)

    in_maps = []
    for c in range(N_CORES):
        cols = []
        for lh in range(HL):
            h = HL * c + lh
            cols.append(h * HD + perm)
        qcols = np.concatenate(cols)
        wq_c = Wq[:, qcols] * scale
        wk_c = Wk[:, qcols]
        wqk_c = np.ascontiguousarray(
            np.concatenate([wq_c, wk_c], axis=1)
        ).astype(mdt_np)  # [C, 512]
        wv_c = np.ascontiguousarray(
            Wv[:, HL * HD * c : HL * HD * (c + 1)]
        ).astype(mdt_np)  # [C, 256]
        in_maps.append(
            {
                "xT": xT,
                "wqk": wqk_c,
                "wv": wv_c,
                "wproj": Wproj_m,
                "cos": cos,
                "sin": sin,
            }
        )
    return in_maps


_NC_CACHE = None


def _get_nc():
    global _NC_CACHE
    if _NC_CACHE is None:
        _NC_CACHE = build_program()
    return _NC_CACHE


def run(x, rope, Wqkv, Wproj, trace=False):
    _install_ntff_shim()
    from concourse.bass_utils import run_bass_kernel_spmd

    nc = _get_nc()
    in_maps = _prep_inputs(x, rope, Wqkv, Wproj)
    res = run_bass_kernel_spmd(nc, in_maps, list(range(N_CORES)), trace=trace)
    # core c holds rows [b*TB:(b+1)*TB] = tokens [c*TB,(c+1)*TB) of batch b
    full = np.zeros((B, T, C), dtype=np.float32)
    for c in range(N_CORES):
        o = res.results[c]["out"].reshape(B, TB, C)
        full[:, c * TB : (c + 1) * TB, :] = o
    return full, res


def kernel(x, rope, Wqkv, Wproj):
    out, _ = run(x, rope, Wqkv, Wproj, trace=False)
    return out


if __name__ == "__main__":
    import time

    t0 = time.time()
    nc = build_program()
    ni = sum(len(bb.instructions) for f in nc.m.functions for bb in f.blocks)
    print(f"build ok: {time.time()-t0:.1f}s, {ni} instructions")


# revision 8
# speedup vs baseline: 1.2100x; 1.1080x over previous
"""Causal self-attention (B=4, T=2048, C=2048, H=16) on 8 trn2 NeuronCores.

Sharding: tensor-parallel over heads (2 heads/core). Each core computes the
QKV projection for its head shard (q,k produced transposed for the scores
matmul, v produced in normal layout for attn@v), applies rope fused into the
PSUM->SBUF drain, runs causal attention without max-subtraction (scores are
O(5), exp is fp32-safe), and produces yT = (attn @ v)^T per head.

All matmuls are bf16. The AllToAll that re-shards Y^T from head-column-
sharded to token-row-sharded is split into FOUR per-batch collectives, each
fired as soon as that batch's attention is done; the output projection for
batch b is emitted during batch b+1's compute so the collective latency is
hidden. Each core ends up with tokens [c*256, (c+1)*256) of every batch
(re-assembled on host). Wproj is resident in SBUF for the whole kernel.

Softmax: denominator accumulated on the PE as an all-ones [128,128] matmul
(output pre-broadcast across partitions), reciprocal on the DVE, normalize
on the DVE straight out of PSUM. All weight tensors are host-pre-transposed
to partition-major layout so their DMAs are ~128 descriptors."""

import os
import sys

os.environ.setdefault("JAX_PLATFORMS", "axon")

import numpy as np

B, T, C = 4, 2048, 2048
H = 16
HD = 128
N_CORES = 8
HL = H // N_CORES  # heads per core = 2
CL = HL * HD  # per-core head columns = 256
TQ = 512  # Tq chunk for scores
NKT = T // 128  # 16 tiles of 128 along T
KC = C // 128  # 16 k-tiles along C
TB = T // N_CORES  # tokens per (batch, core) after re-shard = 256
ROWS = B * TB  # output rows per core = 1024


def _install_ntff_shim():
    """The agent image's antenv lacks axon_hooks; provide it so
    run_bass_kernel_spmd(trace=True) can reach the NTFF profiler."""
    import types, contextlib, ctypes

    try:
        from antenv.axon_hooks import get_axon_ntff_profile_hook  # noqa

        return
    except ImportError:
        pass

    so_path = "/opt/axon/libaxon_pjrt.so"
    try:
        lib = ctypes.CDLL(so_path)
    except OSError:
        lib = None
    if lib is None or not hasattr(lib, "axon_start_nrt_profile"):
        hook = None
    else:
        lib.axon_start_nrt_profile.argtypes = [
            ctypes.POINTER(ctypes.c_int64),
            ctypes.c_size_t,
        ]
        lib.axon_start_nrt_profile.restype = ctypes.c_int64
        lib.axon_stop_nrt_profile.argtypes = [ctypes.c_char_p]
        lib.axon_stop_nrt_profile.restype = ctypes.c_int64

        @contextlib.contextmanager
        def hook(output_dir, device_ids):
            import jax

            jax.devices()
            if device_ids:
                ids = (ctypes.c_int64 * len(device_ids))(*device_ids)
                rc = lib.axon_start_nrt_profile(ids, len(device_ids))
            else:
                rc = lib.axon_start_nrt_profile(None, 0)
            if rc != 0:
                raise RuntimeError(f"axon_start_nrt_profile rc={rc}")
            try:
                yield
            finally:
                n = lib.axon_stop_nrt_profile(str(output_dir).encode())
                if n <= 0:
                    print(f"ntff profile: rc={n} (no files) dir={output_dir}")

    import antenv

    mod = types.ModuleType("antenv.axon_hooks")
    _state = {"hook": hook}
    mod.set_axon_ntff_profile_hook = lambda h: _state.__setitem__("hook", h)
    mod.get_axon_ntff_profile_hook = lambda: _state["hook"]
    sys.modules["antenv.axon_hooks"] = mod
    antenv.axon_hooks = mod


def build_program():
    import concourse.bass as bass
    import concourse.mybir as mybir
    import concourse.tile as tile
    from concourse import bacc
    from contextlib import ExitStack

    f32 = mybir.dt.float32
    f32r = mybir.dt.float32r
    bf16 = mybir.dt.bfloat16
    mdt = bf16
    Exp = mybir.ActivationFunctionType.Exp

    nc = bacc.Bacc("TRN2", target_bir_lowering=False, debug=False, num_devices=N_CORES)

    # all weights host-pre-transposed to partition-major [128, KC, *]
    xT = nc.dram_tensor("xT", [B, C, T], mdt, kind="ExternalInput")
    wqk = nc.dram_tensor("wqk", [128, KC, 4 * HD], mdt, kind="ExternalInput")
    wv = nc.dram_tensor("wv", [128, KC, CL], mdt, kind="ExternalInput")
    wproj = nc.dram_tensor("wproj", [128, KC, C], mdt, kind="ExternalInput")
    cosd = nc.dram_tensor("cos", [HD // 2, T], f32, kind="ExternalInput")
    sind = nc.dram_tensor("sin", [HD // 2, T], f32, kind="ExternalInput")
    out = nc.dram_tensor("out", [ROWS, C], f32, kind="ExternalOutput")

    # shard-major a2a buffers: [src/dest, p, lh, t]
    a2a_in = [
        nc.dram_tensor(f"a2a_in{b}", [N_CORES, 128, HL, TB], mdt) for b in range(B)
    ]
    a2a_out = [
        nc.dram_tensor(f"a2a_out{b}", [N_CORES, 128, HL, TB], mdt) for b in range(B)
    ]

    with tile.TileContext(nc) as tc:
        with ExitStack() as top:
            const = top.enter_context(tc.tile_pool(name="const", bufs=1))
            wpool = top.enter_context(tc.tile_pool(name="weights", bufs=1))

            # --- weights first on the sync queue so MMs can start early ---
            wqk_sb = wpool.tile([128, KC, 4 * HD], mdt, tag="wqk")
            nc.sync.dma_start(wqk_sb[:, 0:4, :], wqk[:, 0:4, :])

            # --- constants ---
            ones_f = const.tile([128, 128], f32, tag="ones_f")
            nc.vector.memset(ones_f[:], 1.0)
            ones128 = const.tile([128, 128], mdt, tag="ones128")
            nc.vector.tensor_copy(ones128[:], ones_f[:])

            cos_sb = const.tile([64, T], f32, tag="cos")
            nc.sync.dma_start(cos_sb[:], cosd[:, :])
            sin_sb = const.tile([64, T], f32, tag="sin")
            nc.sync.dma_start(sin_sb[:], sind[:, :])

            # wproj via the scalar queue so it doesn't delay the xk stream
            wproj_sb = wpool.tile([128, KC, C], mdt, tag="wproj")
            nc.scalar.dma_start(wproj_sb[:], wproj[:, :, :])
            wv_sb = wpool.tile([128, KC, CL], mdt, tag="wv")

            with ExitStack() as mid:
                qk_pool = mid.enter_context(tc.tile_pool(name="qkT", bufs=2))
                v_pool = mid.enter_context(tc.tile_pool(name="vsb", bufs=2))
                xk_pool = mid.enter_context(tc.tile_pool(name="xk", bufs=18))
                rtmp = mid.enter_context(tc.tile_pool(name="rtmp", bufs=1))
                apool = mid.enter_context(tc.tile_pool(name="apool", bufs=3))
                spool = mid.enter_context(tc.tile_pool(name="spool", bufs=2))
                ypool = mid.enter_context(tc.tile_pool(name="yproj", bufs=1))
                opool = mid.enter_context(tc.tile_pool(name="osb", bufs=2))
                ps_mm = mid.enter_context(
                    tc.tile_pool(name="psmm", bufs=3, space="PSUM")
                )
                ps_yT = mid.enter_context(
                    tc.tile_pool(name="psyT", bufs=2, space="PSUM")
                )
                ps_den = mid.enter_context(
                    tc.tile_pool(name="psden", bufs=2, space="PSUM")
                )
                ps_x = mid.enter_context(
                    tc.tile_pool(name="psx", bufs=1, space="PSUM")
                )

                def emit_proj(b, y_sb):
                    for n in range(C // TQ):
                        for m in range(TB // 128):
                            o_ps = ps_x.tile([128, TQ], f32, tag="x")
                            for k in range(KC):
                                nc.tensor.matmul(
                                    o_ps[:],
                                    y_sb[:, k, 128 * m : 128 * (m + 1)],
                                    wproj_sb[:, k, TQ * n : TQ * (n + 1)],
                                    start=(k == 0),
                                    stop=(k == KC - 1),
                                )
                            o_sb = opool.tile([128, TQ], f32, tag="o_sb")
                            nc.scalar.copy(o_sb[:], o_ps[:])
                            nc.sync.dma_start(
                                out[
                                    TB * b + 128 * m : TB * b + 128 * (m + 1),
                                    TQ * n : TQ * (n + 1),
                                ],
                                o_sb[:],
                            )

                y_prev = None
                for b in range(B):
                    # ---------- QKV projection for batch b ----------
                    # qkT [128, 4, T]: m=0,1 -> qT heads 0,1 (rope+scale),
                    # m=2,3 -> kT heads 0,1 (rope). v_sb [128, NKT, CL].
                    qkT = qk_pool.tile([128, 4, T], mdt, tag="qkT")
                    v_sb = v_pool.tile([128, NKT, CL], mdt, tag="v")

                    for n in range(T // TQ):
                        xk = [
                            xk_pool.tile([128, TQ], mdt, tag="xk", name=f"xk{k}")
                            for k in range(KC)
                        ]
                        for k in range(KC):
                            nc.sync.dma_start(
                                xk[k][:],
                                xT[b, 128 * k : 128 * (k + 1), TQ * n : TQ * (n + 1)],
                            )
                        if b == 0 and n == 0:
                            # stream in the rest of the weights behind the
                            # first chunk's xk tiles
                            for kw in range(1, 4):
                                nc.sync.dma_start(
                                    wqk_sb[:, 4 * kw : 4 * (kw + 1), :],
                                    wqk[:, 4 * kw : 4 * (kw + 1), :],
                                )
                            nc.sync.dma_start(wv_sb[:], wv[:, :, :])
                        for m in range(4):
                            qk_ps = ps_mm.tile([128, TQ], f32, tag="mm")
                            for k in range(KC):
                                nc.tensor.matmul(
                                    qk_ps[:],
                                    wqk_sb[:, k, 128 * m : 128 * (m + 1)],
                                    xk[k][:],
                                    start=(k == 0),
                                    stop=(k == KC - 1),
                                )
                            # rope on the PSUM->SBUF drain
                            cos_t = cos_sb[:, TQ * n : TQ * (n + 1)]
                            sin_t = sin_sb[:, TQ * n : TQ * (n + 1)]
                            t0 = rtmp.tile([64, TQ], f32, tag="t0")
                            t1 = rtmp.tile([64, TQ], f32, tag="t1")
                            nc.vector.tensor_mul(t0[:], qk_ps[0:64, :], cos_t)
                            nc.vector.tensor_mul(t1[:], qk_ps[64:128, :], sin_t)
                            nc.vector.tensor_sub(
                                qkT[0:64, m, TQ * n : TQ * (n + 1)], t0[:], t1[:]
                            )
                            t2 = rtmp.tile([64, TQ], f32, tag="t2")
                            t3 = rtmp.tile([64, TQ], f32, tag="t3")
                            nc.vector.tensor_mul(t2[:], qk_ps[64:128, :], cos_t)
                            nc.vector.tensor_mul(t3[:], qk_ps[0:64, :], sin_t)
                            nc.vector.tensor_add(
                                qkT[64:128, m, TQ * n : TQ * (n + 1)], t2[:], t3[:]
                            )
                        for m2 in range(4):
                            v_ps = ps_x.tile([128, TQ], f32, tag="x")
                            for k in range(KC):
                                nc.tensor.matmul(
                                    v_ps[:, 0:CL],
                                    xk[k][:, 128 * m2 : 128 * (m2 + 1)],
                                    wv_sb[:, k, :],
                                    start=(k == 0),
                                    stop=(k == KC - 1),
                                )
                            nc.scalar.copy(v_sb[:, 4 * n + m2, :], v_ps[:, 0:CL])

                    # ---------- attention for batch b ----------
                    for h in range(HL):
                        for j in range(T // TQ):
                            ntk = 4 * j + 4  # causal: k-tiles 0..4j+3
                            yT_ps = ps_yT.tile([128, TQ], f32, tag="yT")
                            den_ps = ps_den.tile([128, TQ], f32, tag="den")
                            for i in range(ntk):
                                sT_ps = ps_mm.tile([128, TQ], f32, tag="mm")
                                nc.tensor.matmul(
                                    sT_ps[:],
                                    qkT[:, 2 + h, 128 * i : 128 * (i + 1)],
                                    qkT[:, h, TQ * j : TQ * (j + 1)],
                                    start=True,
                                    stop=True,
                                )
                                a_sb = apool.tile([128, TQ], mdt, tag="a")
                                nc.scalar.activation(a_sb[:], sT_ps[:], Exp)
                                d = i - 4 * j
                                if d >= 0:
                                    # causal: zero where tq < tk (y < x + 128d)
                                    nc.gpsimd.affine_select(
                                        out=a_sb[:],
                                        in_=a_sb[:],
                                        compare_op=mybir.AluOpType.is_ge,
                                        fill=0.0,
                                        base=-128 * d,
                                        pattern=[[1, TQ]],
                                        channel_multiplier=-1,
                                    )
                                nc.tensor.matmul(
                                    den_ps[:],
                                    ones128[:],
                                    a_sb[:],
                                    start=(i == 0),
                                    stop=(i == ntk - 1),
                                )
                                nc.tensor.matmul(
                                    yT_ps[:],
                                    v_sb[:, i, 128 * h : 128 * (h + 1)],
                                    a_sb[:],
                                    start=(i == 0),
                                    stop=(i == ntk - 1),
                                )
                            # chunk epilogue: reciprocal + normalize (the den
                            # matmul already broadcast den to all partitions)
                            rec_c = spool.tile([128, TQ], f32r, tag="rec")
                            with nc.allow_low_precision(reason="softmax recip"):
                                nc.vector.reciprocal(rec_c[:], den_ps[:])
                            yT_sb = spool.tile([128, TQ], mdt, tag="yT_sb")
                            nc.vector.tensor_mul(
                                yT_sb[:], rec_c[:].bitcast(f32), yT_ps[:]
                            )
                            for u in range(2):
                                nc.sync.dma_start(
                                    a2a_in[b][2 * j + u, :, h, :],
                                    yT_sb[:, TB * u : TB * (u + 1)],
                                )
                            if b > 0 and h == 0 and j == 1:
                                # previous batch's a2a landed long ago; pull
                                # its Y^T in for the interleaved projection
                                y_prev = ypool.tile([128, KC, TB], mdt, tag="y")
                                for s in range(N_CORES):
                                    nc.sync.dma_start(
                                        y_prev[:, HL * s : HL * (s + 1), :],
                                        a2a_out[b - 1][s, :, :, :],
                                    )

                    # proj for the previous batch overlaps this batch's tail
                    if b > 0:
                        emit_proj(b - 1, y_prev)

                    # fire this batch's re-shard as soon as its attn is done
                    nc.gpsimd.collective_compute(
                        "AllToAll",
                        mybir.AluOpType.bypass,
                        replica_groups=[list(range(N_CORES))],
                        ins=[a2a_in[b][:, :, :, :]],
                        outs=[a2a_out[b][:, :, :, :]],
                    )

                # ---------- last batch's projection ----------
                y_last = ypool.tile([128, KC, TB], mdt, tag="y")
                for s in range(N_CORES):
                    nc.sync.dma_start(
                        y_last[:, HL * s : HL * (s + 1), :],
                        a2a_out[B - 1][s, :, :, :],
                    )
                emit_proj(B - 1, y_last)

    nc.compile()
    return nc


_PERM = None


def _prep_inputs(x, rope, Wqkv, Wproj):
    """Host-side sharding/layout prep (numpy only)."""
    global _PERM
    if _PERM is None:
        _PERM = np.concatenate([np.arange(0, HD, 2), np.arange(1, HD, 2)])
    perm = _PERM

    import ml_dtypes

    mdt_np = ml_dtypes.bfloat16

    def pmajor(w):
        # [C, M] -> [128, KC, M] partition-major
        m = w.shape[1]
        return np.ascontiguousarray(
            w.reshape(KC, 128, m).transpose(1, 0, 2)
        ).astype(mdt_np)

    x = np.asarray(x, dtype=np.float32)
    xT = np.ascontiguousarray(x.transpose(0, 2, 1)).astype(mdt_np)  # [B, C, T]

    rope = np.asarray(rope, dtype=np.float32)
    cos = np.ascontiguousarray(rope[:, :, 0].T)  # [64, T]
    sin = np.ascontiguousarray(rope[:, :, 1].T)

    Wqkv = np.asarray(Wqkv, dtype=np.float32)
    Wq = Wqkv[:, 0:C]
    Wk = Wqkv[:, C : 2 * C]
    Wv = Wqkv[:, 2 * C : 3 * C]
    scale = 1.0 / np.sqrt(HD)
    Wproj_p = pmajor(np.asarray(Wproj, dtype=np.float32))

    in_maps = []
    for c in range(N_CORES):
        cols = []
        for lh in range(HL):
            h = HL * c + lh
            cols.append(h * HD + perm)
        qcols = np.concatenate(cols)
        wq_c = Wq[:, qcols] * scale
        wk_c = Wk[:, qcols]
        wqk_c = pmajor(np.concatenate([wq_c, wk_c], axis=1))  # [128, KC, 512]
        wv_c = pmajor(Wv[:, HL * HD * c : HL * HD * (c + 1)])  # [128, KC, 256]
        in_maps.append(
            {
                "xT": xT,
                "wqk": wqk_c,
                "wv": wv_c,
                "wproj": Wproj_p,
                "cos": cos,
                "sin": sin,
            }
        )
    return in_maps


_NC_CACHE = None


def _get_nc():
    global _NC_CACHE
    if _NC_CACHE is None:
        _NC_CACHE = build_program()
    return _NC_CACHE


def run(x, rope, Wqkv, Wproj, trace=False):
    _install_ntff_shim()
    from concourse.bass_utils import run_bass_kernel_spmd

    nc = _get_nc()
    in_maps = _prep_inputs(x, rope, Wqkv, Wproj)
    res = run_bass_kernel_spmd(nc, in_maps, list(range(N_CORES)), trace=trace)
    # core c holds rows [b*TB:(b+1)*TB] = tokens [c*TB,(c+1)*TB) of batch b
    full = np.zeros((B, T, C), dtype=np.float32)
    for c in range(N_CORES):
        o = res.results[c]["out"].reshape(B, TB, C)
        full[:, c * TB : (c + 1) * TB, :] = o
    return full, res


def kernel(x, rope, Wqkv, Wproj):
    out, _ = run(x, rope, Wqkv, Wproj, trace=False)
    return out


if __name__ == "__main__":
    import time

    t0 = time.time()
    nc = build_program()
    ni = sum(len(bb.instructions) for f in nc.m.functions for bb in f.blocks)
    print(f"build ok: {time.time()-t0:.1f}s, {ni} instructions")
